# revision 69
# baseline (speedup 1.0000x reference)
"""GCN message-passing kernel for 8 TRN2 NeuronCores.

Model (reference):
  h1 = relu(gcn(x, W1, b1));  h2 = relu(gcn(h1, W2, b2))
  ho = h2 @ (Wout @ Wv).T + (Wout @ bv + bout)     (attention with seq_len=1)
  out = segment_mean(ho, batch) @ fc_w.T + fc_b

Since pooling and the final linear are linear, the V/out projections fold
past the pooling:  out = segment_mean(h2) @ (WcT @ fc_w.T) + const.

Distribution: nodes sharded contiguously (12500/core); edges partitioned by
destination core and sorted by (dst block, src bucket, dst); self-loops are
explicit edges with weight dinv^2.

Layer 1 exploits IN_DIM==1: t_i = sum_e w_e * x[src_e] is a scalar per node.
The host ships one premultiplied value stream (x[src]*w per edge slot, a
fixed KSLOT slots per node); the device reduces it with a single
tensor_reduce, then builds its shard of the (relu(t*W1+b1)@W2) table in bf16
(rows padded to 256B for the gather element size).

The table is laid out bucket-major: bucket q holds quarter q of every
core's shard, so each of 4 AllGathers completes one gather bucket and
overlaps with layer-2 processing of earlier buckets.

Layer 2 gathers table rows per edge with the GPSIMD dma_gather ucode op
(int16 indices within a bucket; edges packed per (group, bucket) into
128-edge tiles — tiles shared by two dst blocks get one S column per
(block, tile) job).  Gather calls round-robin over all 4 SWDGE queues
(the single-queue drain was the dominant serialization).  Per job a
one-hot matrix S[e, dst]*w_e is built on-chip (iota + dual-op
tensor_scalar) and a TensorE matmul S.T @ V accumulates agg[dst, feat]
in PSUM per 128-node block (64 moving columns, 6 PSUM banks rotating).
b2 is added via a rank-1 ones@b2row matmul into the same PSUM group;
relu gives h2, which pooling consumes directly (one-hot matmuls into a
fixed 256-graph window per core); windows are AllGathered and recombined
at compile-time offsets; the tiny fc runs replicated.
"""

import sys
from dataclasses import dataclass, field

import numpy as np

sys.path.insert(0, "/opt/trn_rl_repo")

P = 128


def _ceil(a, b):
    return -(-a // b)


@dataclass
class Cfg:
    N: int = 100000
    G: int = 1000
    HID: int = 64
    OUT: int = 10
    NC: int = 8
    NQ: int = 4            # table buckets == AllGather chunks
    GNI: int = 8192        # max indices per dma_gather instruction
    GRP_TILES: int = 104   # target tiles per L2 gather group
    TROW: int = 128        # padded table row (bf16 elems, 256B)

    @property
    def NPC(self):
        return self.N // self.NC

    @property
    def NB(self):
        return (self.NPC + P - 1) // P

    QB0: int = 24          # blocks in the first table quarter

    @property
    def QB(self):
        # block boundaries of the NQ node-shard quarters; the first quarter
        # is small so its AllGather (which gates the first L2 gathers)
        # completes early.  Each bucket must stay within int16 index range:
        # NC * rows <= 32767.
        rest = self.NB - self.QB0
        b1 = self.QB0 + (rest + 2) // 3
        b2 = self.QB0 + (2 * rest + 1) // 3
        out = [0, self.QB0, b1, b2, self.NB]
        assert all(
            self.NC * (min(out[i + 1] * P, self.NPC) - out[i] * P) <= 32767
            for i in range(self.NQ))
        return out

    @property
    def QS(self):
        # row boundaries of the quarters within a core shard
        return [min(b * P, self.NPC) for b in self.QB]

    @property
    def QR(self):
        qs = self.QS
        return [qs[i + 1] - qs[i] for i in range(self.NQ)]

    @property
    def QOFF(self):
        # row offset of each bucket in the (conceptual) full table
        qr = self.QR
        out = [0]
        for q in range(self.NQ):
            out.append(out[-1] + self.NC * qr[q])
        return out

    @property
    def GR(self):
        return P                   # graphs per final fc block

    @property
    def GPAD(self):
        return 8 * P               # padded graph count for the AllReduce

    @property
    def PWIN(self):
        return 2 * P               # pooled window rows per core


@dataclass
class Meta:
    T2: int = 0
    NCOL: int = 0
    groups: list = field(default_factory=list)  # [(blocks, {r: (t0, nt, loff)})]
    chunks: dict = field(default_factory=dict)  # r -> [(t0, ntiles, [g...])]
    jobs: dict = field(default_factory=dict)    # b -> [(tile, col, bucket)]
    KSLOT: int = 0
    wbase: list = field(default_factory=list)   # pooled window base per core


# --------------------------------------------------------------------------
# Host-side preprocessing
# --------------------------------------------------------------------------

def prep(x, edge_index, batch, cfg: Cfg):
    N, NC, NPC, NB, NQ = cfg.N, cfg.NC, cfg.NPC, cfg.NB, cfg.NQ
    qs = np.asarray(cfg.QS)
    qr = np.asarray(cfg.QR)
    xf = np.asarray(x, np.float32).reshape(-1)
    src = np.asarray(edge_index[0], dtype=np.int64)
    dst = np.asarray(edge_index[1], dtype=np.int64)
    batch = np.asarray(batch, dtype=np.int64)

    deg = np.bincount(dst, minlength=N).astype(np.float64) + 1.0
    dinv = (1.0 / np.sqrt(deg)).astype(np.float32)

    src_all = np.concatenate([src, np.arange(N, dtype=np.int64)])
    dst_all = np.concatenate([dst, np.arange(N, dtype=np.int64)])
    w_all = np.concatenate([dinv[src] * dinv[dst], dinv * dinv]).astype(np.float32)

    # src -> (bucket, idx16)
    s_c = src_all // NPC
    s_i = src_all % NPC
    s_q = np.searchsorted(qs, s_i, side="right") - 1
    s_idx = s_c * qr[s_q] + (s_i - qs[s_q])

    cores = []
    kslot = 0
    for c in range(NC):
        lo, hi = c * NPC, (c + 1) * NPC
        m = (dst_all >= lo) & (dst_all < hi)
        es, ed, ew = src_all[m], (dst_all[m] - lo), w_all[m]
        eq, eidx = s_q[m], s_idx[m]
        # sort by (block, bucket, dst)
        key = (ed // P) * (NQ * NPC) + eq * NPC + ed
        order = np.argsort(key, kind="stable")
        es, ed, ew = es[order], ed[order], ew[order]
        eq, eidx = eq[order], eidx[order]
        cnt_br = np.zeros((NB, NQ), dtype=np.int64)
        np.add.at(cnt_br, (ed // P, eq), 1)
        din = np.bincount(ed, minlength=NPC)
        kslot = max(kslot, int(din.max()))
        cores.append(dict(es=es, ed=ed, ew=ew, eq=eq, eidx=eidx,
                          cnt_br=cnt_br, din=din, lo=lo))

    meta = Meta()
    meta.KSLOT = _ceil(kslot, 4) * 4
    # conservative per-(block,bucket) tile counts used only to size groups
    TBR = [[int(max(_ceil(cores[c]["cnt_br"][b][r], P)
                    for c in range(NC)))
            for r in range(NQ)] for b in range(NB)]
    tiles_per_block = [sum(TBR[b]) for b in range(NB)]

    # gather groups of consecutive blocks
    groups = []
    b = 0
    while b < NB:
        blocks = [b]
        tot = tiles_per_block[b]
        b += 1
        while b < NB and tot + tiles_per_block[b] <= cfg.GRP_TILES:
            per_r_ok = all(
                sum(TBR[bb][r] for bb in blocks + [b]) * P <= cfg.GNI
                for r in range(NQ))
            if not per_r_ok:
                break
            blocks.append(b)
            tot += tiles_per_block[b]
            b += 1
        groups.append(blocks)
    NG = len(groups)
    gid = np.zeros(NB, dtype=np.int64)
    for g, blocks in enumerate(groups):
        for b in blocks:
            gid[b] = g

    # packed (group, bucket) tiling: blocks laid contiguously, padding
    # only at the end of each (group, bucket) run; boundary tiles shared
    # by two blocks get one S column per (block, tile) "job".
    ofs_b = np.zeros((NC, NB, NQ), dtype=np.int64)
    cnt_gr = np.zeros((NC, NG, NQ), dtype=np.int64)
    for c in range(NC):
        cb = cores[c]["cnt_br"]
        for g, blocks in enumerate(groups):
            for r in range(NQ):
                o = 0
                for b in blocks:
                    ofs_b[c, b, r] = o
                    o += cb[b, r]
                cnt_gr[c, g, r] = o
    ntile_gr = np.zeros((NG, NQ), dtype=np.int64)
    for g in range(NG):
        for r in range(NQ):
            ntile_gr[g, r] = _ceil(int(cnt_gr[:, g, r].max()), P)
    # bucket(phase)-major global tile ids: all groups' bucket-r tiles are
    # contiguous, so phase-r gathers can be merged into few large calls
    t0_arr = np.zeros((NG, NQ), dtype=np.int64)
    t = 0
    for r in range(NQ):
        for g in range(NG):
            t0_arr[g, r] = t
            t += int(ntile_gr[g, r])
    meta.T2 = t
    gdefs = []
    for g, blocks in enumerate(groups):
        rinfo = {}
        lo = 0
        for r in range(NQ):
            rinfo[r] = (int(t0_arr[g, r]), int(ntile_gr[g, r]), lo)
            lo += int(ntile_gr[g, r])
        gdefs.append((blocks, rinfo))
    meta.groups = gdefs
    # gather chunks per bucket: consecutive groups' tile runs merged up to
    # the per-call index budget
    meta.chunks = {}
    for r in range(NQ):
        lst = []
        cur = None
        for g in range(NG):
            nt = int(ntile_gr[g, r])
            if nt == 0:
                continue
            if cur is not None and (cur[1] + nt) * P <= cfg.GNI:
                cur[1] += nt
                cur[2].append(g)
            else:
                cur = [int(t0_arr[g, r]), nt, [g]]
                lst.append(cur)
        meta.chunks[r] = [(c[0], c[1], list(c[2])) for c in lst]

    # union jobs + column assignment
    tmin_br = np.full((NB, NQ), 1 << 30, dtype=np.int64)
    tmax_br = np.full((NB, NQ), -1, dtype=np.int64)
    for c in range(NC):
        cb = cores[c]["cnt_br"]
        for b in range(NB):
            for r in range(NQ):
                cnt = cb[b, r]
                if cnt == 0:
                    continue
                o = ofs_b[c, b, r]
                tmin_br[b, r] = min(tmin_br[b, r], o // P)
                tmax_br[b, r] = max(tmax_br[b, r], (o + cnt - 1) // P)
    jobs = {b: [] for b in range(NB)}
    colbase = np.full((NB, NQ), -1, dtype=np.int64)
    ncol = 0
    for g, blocks in enumerate(groups):
        for r in range(NQ):
            for b in blocks:
                if tmax_br[b, r] < 0:
                    continue
                colbase[b, r] = ncol
                for tt in range(int(tmin_br[b, r]), int(tmax_br[b, r]) + 1):
                    jobs[b].append((int(t0_arr[g, r] + tt), ncol, r))
                    ncol += 1
    meta.jobs = jobs
    meta.NCOL = ncol

    cnt_g = np.bincount(batch, minlength=cfg.G).astype(np.float32)
    inv_cnt_full = (1.0 / np.maximum(cnt_g, 1.0)).astype(np.float32)
    meta.wbase = [c * (NPC * cfg.G // N) - 64 for c in range(NC)]

    per_core = []
    for c in range(NC):
        d = cores[c]
        es, ed, ew, cnt_br = d["es"], d["ed"], d["ew"], d["cnt_br"]
        M = len(es)
        blk, eq = ed // P, d["eq"]

        # re-sort edges by (group, bucket, block, dst)
        key = ((gid[blk] * NQ + eq) * NB + blk) * NPC + ed
        order = np.argsort(key, kind="stable")
        es, ed, ew, eq = es[order], ed[order], ew[order], eq[order]
        eidx = d["eidx"][order]
        blk = ed // P

        # ---- L2 streams
        l2_w = np.zeros((P, meta.NCOL), dtype=np.float32)
        l2_off = np.full((P, meta.NCOL), -1.0, dtype=np.float32)
        l2_idx = np.zeros((P, meta.T2 * 8), dtype=np.int16)  # [128, T2*8]
        # run starts per (block, bucket) in the sorted edge order
        run_start = np.zeros((NB, NQ), dtype=np.int64)
        pos = 0
        for g, blocks in enumerate(groups):
            for r in range(NQ):
                for b in blocks:
                    run_start[b, r] = pos
                    pos += cnt_br[b, r]
        assert pos == M
        # slot within the (group, bucket) run
        jw = ofs_b[c, blk, eq] + (np.arange(M) - run_start[blk, eq])
        gtile = t0_arr[gid[blk], eq] + jw // P
        gp = jw % P
        col = colbase[blk, eq] + (gtile - (t0_arr[gid[blk], eq]
                                           + tmin_br[blk, eq]))
        l2_w[gp, col] = ew
        l2_off[gp, col] = (ed % P).astype(np.float32)
        # int16 idx: j-th index of a gather section; global j = gtile*128+gp
        jglob = gtile * P + gp
        v16 = eidx.astype(np.int16)
        icol = (jglob // 16)
        irow = (jglob % 16)
        for k in range(8):
            l2_idx[irow + 16 * k, icol] = v16

        # ---- L1 value stream: node (b, p) -> cols [b*KSLOT, b*KSLOT+din)
        din = d["din"]
        l1_v = np.zeros((P, NB * meta.KSLOT), dtype=np.float32)
        order1 = np.argsort(ed, kind="stable")
        es1, ed1, ew1 = es[order1], ed[order1], ew[order1]
        estart = np.concatenate([[0], np.cumsum(din)])
        j1 = np.arange(M) - estart[ed1]
        vp = ed1 % P
        vc = (ed1 // P) * meta.KSLOT + j1
        l1_v[vp, vc] = xf[es1] * ew1

        # ---- pooling
        gnode = batch[d["lo"]:d["lo"] + NPC]
        wb = meta.wbase[c]
        assert gnode[0] >= wb and gnode[-1] < wb + cfg.PWIN, \
            f"core {c}: graphs [{gnode[0]},{gnode[-1]}] outside window {wb}"
        pool_go = np.full((P, 2 * NB), -1.0, dtype=np.float32)
        for b in range(NB):
            sl = slice(b * P, min((b + 1) * P, NPC))
            n = sl.stop - sl.start
            for gb in range(2):
                off = gnode[sl] - wb - gb * P
                ok = (off >= 0) & (off < P)
                tmp = np.full(n, -1.0, dtype=np.float32)
                tmp[ok] = off[ok].astype(np.float32)
                pool_go[:n, 2 * b + gb] = tmp
        inv_cnt = np.zeros((P, 8), dtype=np.float32)
        for kblk in range(8):
            rows = min(cfg.GR, cfg.G - kblk * cfg.GR)
            inv_cnt[:rows, kblk] = inv_cnt_full[kblk * cfg.GR:kblk * cfg.GR + rows]

        per_core.append(dict(l2_idx=l2_idx, l2_w=l2_w, l2_off=l2_off,
                             l1_v=l1_v, pool_go=pool_go, inv_cnt=inv_cnt))
    return meta, per_core


# --------------------------------------------------------------------------
# Numpy emulation of the device dataflow (validates prep/layout)
# --------------------------------------------------------------------------

def emulate(inputs, meta: Meta, per_core, cfg: Cfg, bf16=True):
    def to_bf(a):
        if not bf16:
            return a
        import ml_dtypes
        return a.astype(ml_dtypes.bfloat16).astype(np.float32)

    HID, NPC, NB, NQ = cfg.HID, cfg.NPC, cfg.NB, cfg.NQ
    qs, qr, qoff = cfg.QS, cfg.QR, cfg.QOFF
    W1 = np.asarray(inputs["W1"], np.float32).reshape(1, HID)
    b1 = np.asarray(inputs["b1"], np.float32).reshape(HID)
    W2 = np.asarray(inputs["W2"], np.float32)
    b2 = np.asarray(inputs["b2"], np.float32).reshape(HID)
    Wv = np.asarray(inputs["in_proj_w"], np.float32)[2 * HID:]
    bv = np.asarray(inputs["in_proj_b"], np.float32).reshape(-1)[2 * HID:]
    Wout = np.asarray(inputs["out_proj_w"], np.float32)
    bout = np.asarray(inputs["out_proj_b"], np.float32).reshape(-1)
    fcw = np.asarray(inputs["fc_w"], np.float32)
    fcb = np.asarray(inputs["fc_b"], np.float32).reshape(-1)

    Wcombo = to_bf(Wv.T @ (Wout.T @ fcw.T))
    bo = Wout @ bv + bout
    bofc = to_bf(bo @ fcw.T + fcb)

    # full table, bucket-major layout
    tab = np.zeros((cfg.N, HID), dtype=np.float32)
    for c in range(cfg.NC):
        d = per_core[c]
        t = d["l1_v"].reshape(P, NB, meta.KSLOT).sum(-1)
        # node n = b*128+p at t[p, b]
        tn = t.T.reshape(-1)[:NPC]
        h1 = np.maximum(np.outer(tn, W1[0]) + b1[None, :], 0.0)
        rows = to_bf(to_bf(h1) @ to_bf(W2).astype(np.float32))
        for q in range(NQ):
            tab[qoff[q] + c * qr[q]: qoff[q] + (c + 1) * qr[q]] = \
                rows[qs[q]:qs[q + 1]]

    agout = np.zeros((cfg.NC * cfg.PWIN, HID), dtype=np.float32)
    for c in range(cfg.NC):
        d = per_core[c]
        idx = np.zeros(meta.T2 * P, dtype=np.int64)
        for j in range(0, meta.T2 * P, 16):
            idx[j:j + 16] = d["l2_idx"][0:16, j // 16]
        part = np.zeros((cfg.PWIN, HID), np.float32)
        for b in range(NB):
            agg = np.zeros((P, HID), dtype=np.float32)
            for tg, col, r in meta.jobs[b]:
                rows_g = idx[tg * P:(tg + 1) * P] + qoff[r]
                rows_g = np.minimum(rows_g, cfg.N - 1)
                V = to_bf(tab[rows_g])
                w = to_bf(d["l2_w"][:, col])
                off = d["l2_off"][:, col]
                S = (np.arange(P)[None, :] == off[:, None]) * w[:, None]
                agg += S.astype(np.float32).T @ V
            h2 = to_bf(np.maximum(agg + b2[None, :], 0.0))
            for gb in range(2):
                off = d["pool_go"][:, 2 * b + gb]
                S = (np.arange(P)[None, :] == off[:, None]).astype(np.float32)
                part[gb * P:(gb + 1) * P] += S.T @ h2
        agout[c * cfg.PWIN:(c + 1) * cfg.PWIN] = part

    out = np.zeros((cfg.G, cfg.OUT), dtype=np.float32)
    for kblk in range(8):
        rows = min(cfg.GR, cfg.G - kblk * cfg.GR)
        acc = np.zeros((rows, HID), np.float32)
        for c in range(cfg.NC):
            wb = meta.wbase[c]
            lo = kblk * cfg.GR
            s0 = lo - wb
            a = max(0, -s0)
            bnd = min(rows, cfg.PWIN - s0)
            if bnd > a:
                acc[a:bnd] += agout[c * cfg.PWIN + s0 + a:
                                    c * cfg.PWIN + s0 + bnd]
        ic = per_core[0]["inv_cnt"][:rows, kblk]
        pm = to_bf(acc * ic[:, None])
        out[kblk * cfg.GR:kblk * cfg.GR + rows] = \
            pm @ Wcombo.astype(np.float32) + bofc[None, :]
    return out


# --------------------------------------------------------------------------
# Bass kernel builder
# --------------------------------------------------------------------------

def build(meta: Meta, cfg: Cfg, ablate=frozenset(), tune=None):
    import concourse.bass as bass
    import concourse.mybir as mybir
    from concourse import bacc, tile
    from concourse.masks import make_identity

    tune = {**dict(single_packet=False, nqueues=4, phased=False, gbufs=2,
                   aggbufs=6, sbufs=32, h2bufs=4, tail2=False, l1bf=False),
            **(tune or {})}

    f32 = mybir.dt.float32
    bf16 = mybir.dt.bfloat16
    i16 = mybir.dt.int16
    HID, OUT, NB, NPC, NQ = cfg.HID, cfg.OUT, cfg.NB, cfg.NPC, cfg.NQ
    TROW = cfg.TROW
    qs, qr, qoff = cfg.QS, cfg.QR, cfg.QOFF
    QB = cfg.QB

    nc = bacc.Bacc(None, target_bir_lowering=False, debug=False,
                   num_swdge_queues=tune["nqueues"])

    def inp(name, shape, dtype=f32):
        return nc.dram_tensor(name, shape, dtype, kind="ExternalInput")

    W1_d = inp("W1", [1, HID])
    b1_d = inp("b1", [HID, 1])
    W2_d = inp("W2", [HID, HID])
    b2_d = inp("b2", [1, HID])
    inw_d = inp("in_proj_w", [3 * HID, HID])
    inb_d = inp("in_proj_b", [3 * HID, 1])
    outw_d = inp("out_proj_w", [HID, HID])
    outb_d = inp("out_proj_b", [HID, 1])
    fcw_d = inp("fc_w", [OUT, HID])
    fcb_d = inp("fc_b", [1, OUT])
    l2_idx_d = inp("l2_idx", [P, meta.T2 * 8], i16)
    l2_w_d = inp("l2_w", [P, meta.NCOL])
    l2_off_d = inp("l2_off", [P, meta.NCOL])
    l1_v_d = inp("l1_v", [P, NB * meta.KSLOT])
    pool_go_d = inp("pool_go", [P, 2 * NB])
    inv_cnt_d = inp("inv_cnt", [P, 8])
    out_d = nc.dram_tensor("out", [cfg.G, OUT], f32, kind="ExternalOutput")

    with tile.TileContext(nc) as tc:
        with (
            tc.tile_pool(name="const", bufs=1) as cpool,
            tc.tile_pool(name="stream", bufs=1) as spool,
            tc.tile_pool(name="work", bufs=3) as wpool,
            tc.tile_pool(name="stile", bufs=16) as stpool,
            tc.tile_pool(name="gath", bufs=tune["gbufs"]) as gpool,
            tc.tile_pool(name="dram", bufs=1, space="DRAM") as dpool,
        ):
            # ---------- constants & weights ----------
            ident = cpool.tile([P, P], f32)
            make_identity(nc, ident[:])
            ident_bf = cpool.tile([P, P], bf16)
            nc.vector.tensor_copy(ident_bf[:], ident[:])
            iota_bf = cpool.tile([P, P], bf16)
            nc.gpsimd.iota(iota_bf[:], pattern=[[1, P]], base=0,
                           channel_multiplier=0,
                           allow_small_or_imprecise_dtypes=True)
            ones_bf = cpool.tile([1, P], bf16)
            nc.vector.memset(ones_bf[:], 1.0)

            W1_s = cpool.tile([1, HID], f32)
            nc.sync.dma_start(W1_s[:], W1_d[:, :])
            b1_s = cpool.tile([HID, 1], f32)
            nc.sync.dma_start(b1_s[:], b1_d[:, :])
            b2row_s = cpool.tile([1, HID], f32)
            nc.sync.dma_start(b2row_s[:], b2_d[:, :])
            b2row_bf = cpool.tile([1, HID], bf16)
            nc.vector.tensor_copy(b2row_bf[:], b2row_s[:])
            Wv_s = cpool.tile([HID, HID], f32)
            nc.sync.dma_start(Wv_s[:], inw_d[2 * HID:3 * HID, :])
            bv_s = cpool.tile([HID, 1], f32)
            nc.sync.dma_start(bv_s[:], inb_d[2 * HID:3 * HID, :])
            Wout_s = cpool.tile([HID, HID], f32)
            nc.sync.dma_start(Wout_s[:], outw_d[:, :])
            bout_s = cpool.tile([HID, 1], f32)
            nc.sync.dma_start(bout_s[:], outb_d[:, :])
            fcw_s = cpool.tile([OUT, HID], f32)
            nc.sync.dma_start(fcw_s[:], fcw_d[:, :])
            fcb_s = cpool.tile([1, OUT], f32)
            nc.sync.dma_start(fcb_s[:], fcb_d[:, :])
            W2_s = cpool.tile([HID, HID], f32)
            nc.sync.dma_start(W2_s[:], W2_d[:, :])
            W2_bf = cpool.tile([HID, HID], bf16)
            nc.vector.tensor_copy(W2_bf[:], W2_s[:])

            with tc.tile_pool(name="psW", bufs=2, space="PSUM") as psW:
                tp = psW.tile([HID, HID], f32, space="PSUM", tag="w")
                nc.tensor.transpose(tp[:], Wout_s[:], ident[:HID, :HID])
                WoutT_s = cpool.tile([HID, HID], f32)
                nc.vector.tensor_copy(WoutT_s[:], tp[:])
                tp4 = psW.tile([HID, OUT], f32, space="PSUM", tag="w")
                nc.tensor.transpose(tp4[:], fcw_s[:], ident[:OUT, :OUT])
                fcwT_f = cpool.tile([HID, OUT], f32)
                nc.vector.tensor_copy(fcwT_f[:], tp4[:])
                # Wcombo = Wv.T @ Wout.T @ fcw.T
                tpA = psW.tile([HID, OUT], f32, space="PSUM", tag="w")
                nc.tensor.matmul(out=tpA[:], lhsT=Wout_s[:], rhs=fcwT_f[:],
                                 start=True, stop=True)
                A1_sb = cpool.tile([HID, OUT], f32)
                nc.vector.tensor_copy(A1_sb[:], tpA[:])
                tpB = psW.tile([HID, OUT], f32, space="PSUM", tag="w")
                nc.tensor.matmul(out=tpB[:], lhsT=Wv_s[:], rhs=A1_sb[:],
                                 start=True, stop=True)
                Wcombo_bf = cpool.tile([HID, OUT], bf16)
                nc.vector.tensor_copy(Wcombo_bf[:], tpB[:])
                # bofc = (Wout @ bv + bout) @ fcw.T + fcb
                tp3 = psW.tile([HID, 1], f32, space="PSUM", tag="w")
                nc.tensor.matmul(out=tp3[:], lhsT=WoutT_s[:], rhs=bv_s[:],
                                 start=True, stop=True)
                bo_s = cpool.tile([HID, 1], f32)
                nc.vector.tensor_tensor(out=bo_s[:], in0=tp3[:],
                                        in1=bout_s[:],
                                        op=mybir.AluOpType.add)
                tp5 = psW.tile([1, OUT], f32, space="PSUM", tag="w")
                nc.tensor.matmul(out=tp5[:], lhsT=bo_s[:], rhs=fcwT_f[:],
                                 start=True, stop=True)
                bofc_f = cpool.tile([1, OUT], f32)
                nc.vector.tensor_tensor(out=bofc_f[:], in0=tp5[:],
                                        in1=fcb_s[:],
                                        op=mybir.AluOpType.add)
                bofc_s = cpool.tile([1, OUT], bf16)
                nc.vector.tensor_copy(bofc_s[:], bofc_f[:])

            # ---------- streams ----------
            l1_v_s = spool.tile([P, NB * meta.KSLOT], f32)
            nc.sync.dma_start(l1_v_s[:], l1_v_d[:, :])
            l2_idx_s = spool.tile([P, meta.T2 * 8], i16)
            if tune["l1bf"]:
                # per-bucket chunks so the first gather's desc-gen only
                # waits for the bucket-0 slice of the index stream
                for r in range(NQ):
                    b0 = meta.groups[0][1][r][0]
                    b1 = (meta.groups[-1][1][r][0]
                          + meta.groups[-1][1][r][1])
                    nc.sync.dma_start(l2_idx_s[:, b0 * 8:b1 * 8],
                                      l2_idx_d[:, b0 * 8:b1 * 8])
            else:
                nc.sync.dma_start(l2_idx_s[:], l2_idx_d[:, :])
            l2_w_s = spool.tile([P, meta.NCOL], f32)
            nc.sync.dma_start(l2_w_s[:], l2_w_d[:, :])
            l2_off_s = spool.tile([P, meta.NCOL], f32)
            nc.sync.dma_start(l2_off_s[:], l2_off_d[:, :])
            pool_go_s = spool.tile([P, 2 * NB], f32)
            nc.sync.dma_start(pool_go_s[:], pool_go_d[:, :])
            inv_cnt_s = spool.tile([P, 8], f32)
            nc.sync.dma_start(inv_cnt_s[:], inv_cnt_d[:, :])

            # ---------- DRAM temporaries ----------
            h1_mine = [dpool.tile([qr[q], TROW], bf16, name=f"h1m{q}")
                       for q in range(NQ)]
            h1_tab = [dpool.tile([cfg.NC * qr[q], TROW], bf16,
                                 addr_space="Shared", name=f"h1t{q}")
                      for q in range(NQ)]
            ag_in = dpool.tile([cfg.PWIN, HID], f32)
            ag_out = dpool.tile([cfg.NC * cfg.PWIN, HID], f32,
                                addr_space="Shared")

            # ---------- Layer 1: t = reduce(l1_v); table = relu(t*W1+b1)@W2
            with tc.tile_pool(name="ps1", bufs=3, space="PSUM") as ps1:
                t_sb = spool.tile([P, NB], f32)
                nc.vector.tensor_reduce(
                    out=t_sb[:],
                    in_=l1_v_s[:].rearrange("p (b k) -> p b k",
                                            k=meta.KSLOT),
                    axis=mybir.AxisListType.X,
                    op=mybir.AluOpType.add)
                l1dt = bf16 if tune["l1bf"] else f32
                W1_l1 = W1_s
                if tune["l1bf"]:
                    W1_l1 = cpool.tile([1, HID], bf16)
                    nc.vector.tensor_copy(W1_l1[:], W1_s[:])
                tT_p = ps1.tile([P, P], f32, space="PSUM", tag="tt", bufs=1)
                nc.tensor.transpose(tT_p[:NB, :], t_sb[:], ident[:])
                tT_s = spool.tile([NB, P], l1dt)
                nc.vector.tensor_copy(tT_s[:], tT_p[:NB, :])
                t_row = spool.tile([1, NB * P], l1dt)
                nc.sync.dma_start(
                    t_row[0:1, :].rearrange("o (b p) -> o b p", b=NB),
                    tT_s[:, :])

                for q in range(NQ):
                    for b in range(QB[q], QB[q + 1]):
                        rows = min(P, NPC - b * P)
                        hp = ps1.tile([HID, P], f32, space="PSUM", tag="hp",
                                      bufs=3)
                        nc.tensor.matmul(out=hp[:, :rows], lhsT=W1_l1[:],
                                         rhs=t_row[0:1, b * P:b * P + rows],
                                         start=True, stop=True)
                        h1T = wpool.tile([HID, P], bf16, tag="h1T",
                                         name="h1T")
                        nc.scalar.activation(
                            h1T[:, :rows], hp[:, :rows],
                            mybir.ActivationFunctionType.Relu,
                            bias=b1_s[:])
                        hw = ps1.tile([P, HID], f32, space="PSUM", tag="hw",
                                      bufs=3)
                        nc.tensor.matmul(out=hw[:rows, :],
                                         lhsT=h1T[:, :rows], rhs=W2_bf[:],
                                         start=True, stop=True)
                        h1b = wpool.tile([P, HID], bf16, tag="h1b",
                                         name="h1b")
                        nc.vector.tensor_copy(h1b[:rows, :], hw[:rows, :])
                        nc.sync.dma_start(
                            h1_mine[q][b * P - qs[q]:
                                       b * P - qs[q] + rows, :HID],
                            h1b[:rows, :])
                    if "coll" not in ablate:
                        nc.gpsimd.collective_compute(
                            "AllGather", mybir.AluOpType.bypass,
                            replica_groups=[list(range(cfg.NC))],
                            ins=[h1_mine[q][:].opt()],
                            outs=[h1_tab[q][:].opt()],
                        )

            # ---------- Layer 2 + pooling ----------
            def build_s2(cl):
                s2 = stpool.tile([P, P], bf16, tag="sl2", name="sl2",
                                 bufs=tune["sbufs"])
                nc.vector.tensor_scalar(
                    out=s2[:], in0=iota_bf[:],
                    scalar1=l2_off_s[:, cl:cl + 1],
                    scalar2=l2_w_s[:, cl:cl + 1],
                    op0=mybir.AluOpType.is_equal,
                    op1=mybir.AluOpType.mult)
                return s2

            def do_pool(pool_ps, b, h2):
                for gb in range(2):
                    sp2 = stpool.tile([P, P], bf16, tag="spool", name="sp2")
                    nc.vector.tensor_scalar(
                        out=sp2[:], in0=iota_bf[:],
                        scalar1=pool_go_s[:, 2 * b + gb: 2 * b + gb + 1],
                        scalar2=None,
                        op0=mybir.AluOpType.is_equal)
                    nc.tensor.matmul(
                        out=pool_ps[gb][:], lhsT=sp2[:], rhs=h2[:],
                        start=(b == 0), stop=(b == NB - 1))

            gather_seq = [0]

            def issue_gather(g2, out_off, rr, t0, ntile):
                ni = ntile * P
                nc.gpsimd.dma_gather(
                    out_ap=g2[:, out_off * TROW:
                              (out_off + ntile) * TROW]
                    .rearrange("p (c d) -> p c d", d=TROW),
                    in_ap=h1_tab[rr][:, :],
                    idxs_ap=l2_idx_s[:, t0 * 8:(t0 + ntile) * 8],
                    num_idxs=ni, num_idxs_reg=ni, elem_size=TROW,
                    single_packet=tune["single_packet"],
                    queue_num=(gather_seq[0] + gather_seq[0] // 4)
                    % tune["nqueues"])
                gather_seq[0] += 1

            with (
                tc.tile_pool(name="ps2", bufs=4, space="PSUM") as ps2,
                tc.tile_pool(name="psPl", bufs=1, space="PSUM") as psPl,
            ):
                pool_ps = [psPl.tile([P, HID], f32, space="PSUM",
                                     tag=f"pool{gb}", name=f"pool_ps{gb}")
                           for gb in range(2)]

                if not tune["phased"]:
                    for blocks, rinfo in meta.groups:
                        if "l2gather" not in ablate:
                            g2 = gpool.tile([P, cfg.GRP_TILES * TROW], bf16,
                                            tag="g2", name="g2")
                            for r in range(NQ):
                                t0, ntile, loff = rinfo[r]
                                if ntile:
                                    issue_gather(g2, loff, r, t0, ntile)

                        def rhs_of(tg, r):
                            if "l2gather" in ablate:
                                return iota_bf[:, :HID]
                            t0, _, loff = rinfo[r]
                            o = loff + tg - t0
                            return g2[:, o * TROW: o * TROW + HID]

                        for b in blocks:
                            agg = ps2.tile([P, HID], f32, space="PSUM",
                                           tag="agg", name="agg",
                                           bufs=tune["aggbufs"])
                            jobs_b = (meta.jobs[b][:1] if "l2mm" in ablate
                                      else meta.jobs[b])
                            for kk, (tg, cl, r) in enumerate(jobs_b):
                                nc.tensor.matmul(
                                    out=agg[:], lhsT=build_s2(cl)[:],
                                    rhs=rhs_of(tg, r),
                                    start=(kk == 0), stop=False)
                            # + b2 (rank-1), closes the PSUM group
                            nc.tensor.matmul(
                                out=agg[:], lhsT=ones_bf[:], rhs=b2row_bf[:],
                                start=False, stop=True)
                            h2 = wpool.tile([P, HID], bf16, tag="h2",
                                            name="h2", bufs=tune["h2bufs"])
                            nc.scalar.activation(
                                h2[:], agg[:],
                                mybir.ActivationFunctionType.Relu)
                            do_pool(pool_ps, b, h2)
                else:
                    # per-bucket phases with SBUF accumulator: phase r only
                    # waits for AllGather r, so collectives overlap compute;
                    # per phase, gathers are merged into few large calls
                    acc_sb = spool.tile([P, NB * HID], f32)
                    maxct = max(c[1] for r in range(NQ)
                                for c in meta.chunks[r])
                    for r in range(NQ):
                        for ct0, ctiles, gids in meta.chunks[r]:
                            if "l2gather" not in ablate:
                                g2 = gpool.tile([P, maxct * TROW], bf16,
                                                tag="g2p", name="g2p",
                                                bufs=3)
                                issue_gather(g2, 0, r, ct0, ctiles)

                            def rhs_of2(tg):
                                if "l2gather" in ablate:
                                    return iota_bf[:, :HID]
                                o = tg - ct0
                                return g2[:, o * TROW: o * TROW + HID]

                            for g in gids:
                                blocks, rinfo = meta.groups[g]
                                for b in blocks:
                                    jb = [(tg, cl) for (tg, cl, rr)
                                          in meta.jobs[b] if rr == r]
                                    if "l2mm" in ablate:
                                        jb = jb[:1] if r == 0 else []
                                    if not jb and r > 0:
                                        continue
                                    agg = ps2.tile([P, HID], f32,
                                                   space="PSUM",
                                                   tag="agg", name="agg")
                                    for kk, (tg, cl) in enumerate(jb):
                                        nc.tensor.matmul(
                                            out=agg[:],
                                            lhsT=build_s2(cl)[:],
                                            rhs=rhs_of2(tg),
                                            start=(kk == 0),
                                            stop=(r > 0 and
                                                  kk == len(jb) - 1))
                                    if r == 0:
                                        nc.tensor.matmul(
                                            out=agg[:], lhsT=ones_bf[:],
                                            rhs=b2row_bf[:],
                                            start=(len(jb) == 0), stop=True)
                                        nc.vector.tensor_copy(
                                            acc_sb[:, b * HID:
                                                   (b + 1) * HID],
                                            agg[:])
                                    else:
                                        nc.vector.tensor_tensor(
                                            out=acc_sb[:, b * HID:
                                                       (b + 1) * HID],
                                            in0=acc_sb[:, b * HID:
                                                       (b + 1) * HID],
                                            in1=agg[:],
                                            op=mybir.AluOpType.add)
                    for b in range(NB):
                        h2 = wpool.tile([P, HID], bf16, tag="h2", name="h2")
                        nc.scalar.activation(
                            h2[:], acc_sb[:, b * HID:(b + 1) * HID],
                            mybir.ActivationFunctionType.Relu)
                        do_pool(pool_ps, b, h2)

                pooled_w = wpool.tile([P, 2 * HID], f32, tag="pw", name="pw")
                for gb in range(2):
                    nc.vector.tensor_copy(
                        pooled_w[:, gb * HID:(gb + 1) * HID], pool_ps[gb][:])

            nc.sync.dma_start(
                ag_in[:].rearrange("(g p) h -> p g h", p=P),
                pooled_w[:].rearrange("p (g h) -> p g h", g=2))

            # ---------- AllGather pooled windows ----------
            if "coll" not in ablate and "tail" not in ablate:
                nc.gpsimd.collective_compute(
                    "AllGather", mybir.AluOpType.bypass,
                    replica_groups=[list(range(cfg.NC))],
                    ins=[ag_in[:].opt()],
                    outs=[ag_out[:].opt()],
                )

            # ---------- recombine windows + final linear ----------
            # two passes: all segment loads issued first (pipelined DMAs),
            # then the per-kblk compute chains run without DMA stalls
            with tc.tile_pool(name="psF", bufs=2, space="PSUM") as psF:
                nkblk = 8 if "tail" not in ablate else 1

                def load_segs(kblk):
                    rows = min(cfg.GR, cfg.G - kblk * cfg.GR)
                    lo = kblk * cfg.GR
                    segs = []
                    for c in range(cfg.NC):
                        s0 = lo - meta.wbase[c]
                        a = max(0, -s0)
                        bnd = min(rows, cfg.PWIN - s0)
                        if bnd <= a:
                            continue
                        seg = wpool.tile([P, HID], f32, tag="seg",
                                         name="seg", bufs=24)
                        nc.vector.memset(seg[:], 0.0)
                        nc.sync.dma_start(
                            seg[a:bnd, :],
                            ag_out[c * cfg.PWIN + s0 + a:
                                   c * cfg.PWIN + s0 + bnd, :])
                        segs.append(seg)
                    return segs

                seg_of = {}
                if tune["tail2"]:
                    for kblk in range(nkblk):
                        seg_of[kblk] = load_segs(kblk)
                for kblk in range(nkblk):
                    if not tune["tail2"]:
                        seg_of[kblk] = load_segs(kblk)
                    rows = min(cfg.GR, cfg.G - kblk * cfg.GR)
                    segs = seg_of[kblk]
                    acc = wpool.tile([P, HID], f32, tag="acc", name="acc")
                    nc.vector.tensor_copy(acc[:], segs[0][:])
                    for seg in segs[1:]:
                        nc.vector.tensor_tensor(
                            out=acc[:rows, :], in0=acc[:rows, :],
                            in1=seg[:rows, :], op=mybir.AluOpType.add)
                    pm = wpool.tile([P, HID], bf16, tag="pm", name="pm")
                    nc.scalar.activation(
                        pm[:rows, :], acc[:rows, :],
                        mybir.ActivationFunctionType.Copy,
                        scale=inv_cnt_s[:rows, kblk:kblk + 1])
                    pmT_p = psF.tile([P, P], bf16, space="PSUM", tag="f")
                    nc.tensor.transpose(pmT_p[:HID, :rows], pm[:rows, :],
                                        ident_bf[:rows, :rows])
                    pmT_s = wpool.tile([HID, P], bf16, tag="pmT_s",
                                       name="pmT_s")
                    nc.vector.tensor_copy(pmT_s[:, :rows], pmT_p[:HID, :rows])
                    op = psF.tile([P, P], f32, space="PSUM", tag="f2")
                    nc.tensor.matmul(out=op[:rows, :OUT],
                                     lhsT=pmT_s[:, :rows],
                                     rhs=Wcombo_bf[:], start=True, stop=False)
                    nc.tensor.matmul(out=op[:rows, :OUT],
                                     lhsT=ones_bf[:, :rows],
                                     rhs=bofc_s[:], start=False, stop=True)
                    ob = wpool.tile([P, OUT], f32, tag="ob", name="ob")
                    nc.vector.tensor_copy(ob[:rows, :], op[:rows, :OUT])
                    nc.sync.dma_start(
                        out_d[kblk * cfg.GR:kblk * cfg.GR + rows, :],
                        ob[:rows, :])

    nc.finalize()
    return nc


# --------------------------------------------------------------------------
# Runner
# --------------------------------------------------------------------------

def make_in_maps(inputs, per_core, cfg: Cfg):
    HID = cfg.HID
    shared = dict(
        W1=np.asarray(inputs["W1"], np.float32).reshape(1, HID),
        b1=np.asarray(inputs["b1"], np.float32).reshape(HID, 1),
        W2=np.asarray(inputs["W2"], np.float32),
        b2=np.asarray(inputs["b2"], np.float32).reshape(1, HID),
        in_proj_w=np.asarray(inputs["in_proj_w"], np.float32),
        in_proj_b=np.asarray(inputs["in_proj_b"], np.float32).reshape(-1, 1),
        out_proj_w=np.asarray(inputs["out_proj_w"], np.float32),
        out_proj_b=np.asarray(inputs["out_proj_b"], np.float32).reshape(-1, 1),
        fc_w=np.asarray(inputs["fc_w"], np.float32),
        fc_b=np.asarray(inputs["fc_b"], np.float32).reshape(1, -1),
    )
    in_maps = []
    for c in range(cfg.NC):
        d = per_core[c]
        m = dict(shared)
        m.update(
            l2_idx=d["l2_idx"], l2_w=d["l2_w"], l2_off=d["l2_off"],
            l1_v=d["l1_v"], pool_go=d["pool_go"], inv_cnt=d["inv_cnt"],
        )
        in_maps.append(m)
    return in_maps


def kernel(x, edge_index, batch, W1, b1, W2, b2, in_proj_w, in_proj_b,
           out_proj_w, out_proj_b, fc_w, fc_b):
    from concourse import bass_utils

    cfg = Cfg()
    inputs = dict(x=x, edge_index=edge_index, batch=batch, W1=W1, b1=b1,
                  W2=W2, b2=b2, in_proj_w=in_proj_w, in_proj_b=in_proj_b,
                  out_proj_w=out_proj_w, out_proj_b=out_proj_b,
                  fc_w=fc_w, fc_b=fc_b)
    meta, per_core = prep(x, edge_index, batch, cfg)
    nc = build(meta, cfg)
    in_maps = make_in_maps(inputs, per_core, cfg)
    res = bass_utils.run_bass_kernel_spmd(
        nc, in_maps, core_ids=list(range(cfg.NC)))
    return np.asarray(res.results[0]["out"], np.float32)
